# revision 17
# baseline (speedup 1.0000x reference)
"""Trainium2 Bass kernel for nn_Blur: depthwise 4x4 FIR blur (upfirdn2d pad=(2,1)).

Full inputs in, full output out. Internally shards the 4096 (b,c) images
across 8 NeuronCores (pure data parallel, no collectives).

v2 (bf16 I/O): the 2e-2 rel-err budget dwarfs bf16 rounding noise (~1.5e-3),
so x is converted to bf16 on the host and the output is written as bf16 and
upcast on the host -- halving HBM traffic, which bound v1.

Layout: the host packs each core's 512 [128,128] images into one gapped
bf16 strip x[h, k*130 + 2 + w] with 2 zero columns between images, so
  - every input DMA is a single fully-contiguous [128, ~3KB/partition]
    transfer (v1 moved 512B descriptors at ~77% efficiency), and
  - the 4 column-shifted accumulating matmuls (j-taps of the separable
    FIR, shift d=j-2 realized by slicing the moving operand) read zeros
    at image edges with no per-group gap-zeroing DMAs.
Per group of 3 images: psum[:, m] += W_j^T @ x[:, m+d], W_j[hi,ho] =
wf[hi-ho+2, j] (wf = flipped kernel) -- contraction over H on the
partition axis, 4 matmuls of ~390 bf16 columns each (1 col/cycle).
PSUM -> SBUF copy de-gaps and casts to bf16; output DMAs are contiguous.
"""

import os
import sys
from contextlib import ExitStack

for _p in ("/opt/trn_rl_repo", "/root/.axon_site/_ro/trn_rl_repo"):
    if os.path.isdir(_p) and _p not in sys.path:
        sys.path.append(_p)

import ml_dtypes
import numpy as np

import concourse.bass as bass  # noqa: F401
import concourse.tile as tile
from concourse import bacc, bass_utils, mybir

B, C, H, W = 16, 256, 128, 128
N_CORES = 8
GROUP = 3          # images per PSUM bank / matmul group (130*3+2 fp32 <= 2KB bank)
STRIDE = 130       # 2-col zero gap + 128 data cols per image in the packed strip
PAD0 = 2           # upfirdn2d pad before (both spatial dims)
SG = 12            # images per DMA supergroup (4 matmul groups)
BF16 = ml_dtypes.bfloat16

# How the odd column shifts (d = j-2 = -1, +1) are realized:
#   "src": shift the moving-operand slice (odd bf16 SBUF start column)
#   "dst": shift the PSUM destination slice (odd fp32 PSUM start column)
SHIFT_MODE = os.environ.get("BLUR_SHIFT_MODE", "src")

_PROGRAM_CACHE: dict[tuple, object] = {}


def _band_matrices(kern: np.ndarray) -> np.ndarray:
    """bands[j][hi, ho] = wf[hi-ho+2, j], wf = flip(kern). Shape [4,128,128]."""
    wf = np.flip(np.asarray(kern, dtype=np.float64), (0, 1))
    bands = np.zeros((4, H, H), dtype=np.float64)
    ho = np.arange(H)
    for j in range(4):
        for i in range(4):
            d = i - PAD0            # hi - ho
            hi = ho + d
            m = (hi >= 0) & (hi < H)
            bands[j][hi[m], ho[m]] = wf[i, j]
    return bands


def _groups(n_images: int):
    """Split a supergroup's images into PSUM-bank groups of <= GROUP."""
    out = []
    i = 0
    while i < n_images:
        n = min(GROUP, n_images - i)
        out.append((i, n))
        i += n
    return out


def _supergroups(n_images: int):
    """Two tiny leading supergroups (fast PE warm-up: the first input DMAs
    land quickly) and a tiny trailing one (short copy+store tail); full-size
    SG in the middle."""
    out = []
    i = 0
    if n_images > 2 * SG:
        out = [(0, GROUP), (GROUP, GROUP)]
        i = 2 * GROUP
    while i < n_images:
        n = min(SG, n_images - i)
        # avoid a final full SG: split off a small trailing one
        if n == SG and n_images - i == SG:
            n = SG - 2
        out.append((i, n))
        i += n
    return out


def pack_input(x_imgs: np.ndarray) -> np.ndarray:
    """[n,128,128] fp32 -> gapped bf16 strip [128, (n+1)*130] (trailing
    block all zero; image k's data at cols [130k+2, 130k+130))."""
    n = x_imgs.shape[0]
    g = np.zeros((H, n + 1, STRIDE), dtype=BF16)
    g[:, :n, PAD0:] = x_imgs.transpose(1, 0, 2)
    return g.reshape(H, (n + 1) * STRIDE)


def build_program(n_images: int, shift_mode: str = SHIFT_MODE):
    nc = bacc.Bacc("TRN2", target_bir_lowering=False, debug=False)
    f32 = mybir.dt.float32
    bf16 = mybir.dt.bfloat16

    x_d = nc.dram_tensor("x", [H, (n_images + 1) * STRIDE], bf16, kind="ExternalInput")
    w_d = nc.dram_tensor("w", [H, 4 * H], bf16, kind="ExternalInput")
    y_d = nc.dram_tensor("y", [H, n_images * W], bf16, kind="ExternalOutput")

    sgs = _supergroups(n_images)

    with ExitStack() as ctx:
        tc = ctx.enter_context(tile.TileContext(nc))
        wpool = ctx.enter_context(tc.tile_pool(name="wpool", bufs=1))
        xpool = ctx.enter_context(tc.tile_pool(name="xpool", bufs=6))
        opool = ctx.enter_context(tc.tile_pool(name="opool", bufs=4))
        ppool = ctx.enter_context(tc.tile_pool(name="ppool", bufs=8, space="PSUM"))

        wt = wpool.tile([H, 4 * H], bf16)
        nc.sync.dma_start(wt, w_d[:, :])

        # PE warm-up: HAM throttles the array to K=4/8 (and low p-state) for
        # the first ~3.4us of activity. Burn that window on dummy matmuls
        # while the PE would otherwise idle waiting for the first DMAs, so
        # the real stream starts at full rate. gpsimd.memset gives an
        # initialized source with no DMA dependency.
        warm = wpool.tile([H, 3 * H], bf16, name="warm", tag="warm")
        nc.gpsimd.memset(warm, 0.0)
        warm_pt = ppool.tile([H, 2 * H], f32, tag="pt", name="warm_pt")
        n_warm = 12
        for i in range(n_warm):
            nc.tensor.matmul(
                warm_pt,
                warm[:, 0:H],
                warm[:, H : 3 * H],
                start=(i == 0),
                stop=(i == n_warm - 1),
            )

        # DMA granularity is decoupled from the PSUM supergroups: one load /
        # one store covers a BLOCK of up to 2 SGs (24 images) -- half the
        # DMAs, half the semaphores the teardown sweep and the issuing
        # engines pay for, same matmul/copy structure.
        blocks = []  # (sg_lo, sg_hi) inclusive
        s = 0
        while s < len(sgs):
            if sgs[s][1] == SG and s + 1 < len(sgs) and sgs[s + 1][1] == SG:
                blocks.append((s, s + 1))
                s += 2
            else:
                blocks.append((s, s))
                s += 1
        sg2block = {}
        for bi, (lo, hi) in enumerate(blocks):
            for ss in range(lo, hi + 1):
                sg2block[ss] = bi

        xts: dict[int, object] = {}
        ots: dict[int, object] = {}

        def in_dma(bi):
            lo, hi = blocks[bi]
            i0 = sgs[lo][0]
            n = sgs[hi][0] + sgs[hi][1] - i0
            xt = xpool.tile([H, 2 * SG * STRIDE + 2], bf16, name="xt", tag="xt")
            nc.sync.dma_start(
                xt[:, 0 : n * STRIDE + 2],
                x_d[:, i0 * STRIDE : i0 * STRIDE + n * STRIDE + 2],
            )
            xts[bi] = xt

        for b0 in range(min(3, len(blocks))):
            in_dma(b0)
        for s, (i0, n) in enumerate(sgs):
            bi = sg2block[s]
            lo, hi = blocks[bi]
            if s == lo and bi + 3 < len(blocks):
                in_dma(bi + 3)
            xt = xts[bi]
            boff = i0 - sgs[lo][0]  # image offset of this SG within its block
            gs = _groups(n)

            pts = [
                ppool.tile([H, ng * STRIDE + 2], f32, tag="pt", name="pt")
                for (g0, ng) in gs
            ]
            for idx, j in enumerate((2, 0, 1, 3)):
                d = j - PAD0
                for q, (g0, ng) in enumerate(gs):
                    base = (boff + g0) * STRIDE
                    if shift_mode == "src" or d in (0, -2):
                        a = PAD0
                        b = ng * STRIDE + PAD0 - (PAD0 if d > 0 else 0)
                        dst = pts[q][:, a:b]
                        src = xt[:, base + a + d : base + b + d]
                    elif d == -1:
                        dst = pts[q][:, 1 : ng * STRIDE + 1]
                        src = xt[:, base : base + ng * STRIDE]
                    else:  # d == +1
                        dst = pts[q][:, 1 : ng * STRIDE + 1]
                        src = xt[:, base + 2 : base + ng * STRIDE + 2]
                    nc.tensor.matmul(
                        dst,
                        wt[:, H * j : H * (j + 1)],
                        src,
                        start=(idx == 0),
                        stop=(idx == 3),
                    )

            # all of one SG's copies on a single engine (alternating per SG):
            # halves the PSUM-bank-free semaphore waits on the Tensor queue
            if bi not in ots:
                ots[bi] = opool.tile([H, 2 * SG * W], bf16, tag="ot", name="ot")
            ot = ots[bi]
            for q, (g0, ng) in enumerate(gs):
                psrc = pts[q][:, 0 : ng * STRIDE].rearrange(
                    "p (k c) -> p k c", c=STRIDE
                )
                odst = ot[:, (boff + g0) * W : (boff + g0 + ng) * W].rearrange(
                    "p (k c) -> p k c", c=W
                )
                if s % 2 == 0:
                    nc.vector.tensor_copy(odst, psrc[:, :, PAD0 : PAD0 + W])
                else:
                    nc.scalar.copy(odst, psrc[:, :, PAD0 : PAD0 + W])
            if s == hi:
                # one store per block; final stores go out on the (idle by
                # then) sync ring so they don't queue behind earlier stores
                # on scalar's FIFO ring
                del xts[bi]
                bi0 = sgs[lo][0]
                bn = i0 + n - bi0
                oeng = nc.sync if bi >= len(blocks) - 2 else nc.scalar
                oeng.dma_start(
                    y_d[:, bi0 * W : (bi0 + bn) * W], ots.pop(bi)[:, 0 : bn * W]
                )

    nc.compile()
    return nc


def _get_program(n_images: int):
    key = (n_images, SHIFT_MODE)
    if key not in _PROGRAM_CACHE:
        _PROGRAM_CACHE[key] = build_program(n_images)
    return _PROGRAM_CACHE[key]


def kernel(x: np.ndarray, kernel: np.ndarray, _trace: bool = False):
    assert x.shape == (B, C, H, W), x.shape
    bands = _band_matrices(kernel)
    wt_host = np.concatenate(list(bands), axis=1).astype(BF16)  # [128, 512]

    n_total = B * C
    n_per_core = n_total // N_CORES
    xb = np.asarray(x, dtype=np.float32).reshape(n_total, H, W).astype(BF16)

    nc = _get_program(n_per_core)
    in_maps = [
        {
            "x": pack_input(xb[c * n_per_core : (c + 1) * n_per_core]),
            "w": wt_host,
        }
        for c in range(N_CORES)
    ]
    res = bass_utils.run_bass_kernel_spmd(
        nc, in_maps, core_ids=list(range(N_CORES)), trace=_trace
    )
    outs = [
        np.asarray(r["y"])
        .reshape(H, n_per_core, W)
        .transpose(1, 0, 2)
        .astype(np.float32)
        for r in res.results
    ]
    y = np.concatenate(outs, axis=0).reshape(B, C, H, W)
    if _trace:
        return y, res
    return y


# revision 19
# speedup vs baseline: 1.0224x; 1.0224x over previous
"""Trainium2 Bass kernel for nn_Blur: depthwise 4x4 FIR blur (upfirdn2d pad=(2,1)).

Full inputs in, full output out. Internally shards the 4096 (b,c) images
across 8 NeuronCores (pure data parallel, no collectives).

v2 (bf16 I/O): the 2e-2 rel-err budget dwarfs bf16 rounding noise (~1.5e-3),
so x is converted to bf16 on the host and the output is written as bf16 and
upcast on the host -- halving HBM traffic, which bound v1.

Layout: the host packs each core's 512 [128,128] images into one gapped
bf16 strip x[h, k*130 + 2 + w] with 2 zero columns between images, so
  - every input DMA is a single fully-contiguous [128, ~3KB/partition]
    transfer (v1 moved 512B descriptors at ~77% efficiency), and
  - the 4 column-shifted accumulating matmuls (j-taps of the separable
    FIR, shift d=j-2 realized by slicing the moving operand) read zeros
    at image edges with no per-group gap-zeroing DMAs.
Per group of 3 images: psum[:, m] += W_j^T @ x[:, m+d], W_j[hi,ho] =
wf[hi-ho+2, j] (wf = flipped kernel) -- contraction over H on the
partition axis, 4 matmuls of ~390 bf16 columns each (1 col/cycle).
PSUM -> SBUF copy de-gaps and casts to bf16; output DMAs are contiguous.
"""

import os
import sys
from contextlib import ExitStack

for _p in ("/opt/trn_rl_repo", "/root/.axon_site/_ro/trn_rl_repo"):
    if os.path.isdir(_p) and _p not in sys.path:
        sys.path.append(_p)

import ml_dtypes
import numpy as np

import concourse.bass as bass  # noqa: F401
import concourse.tile as tile
from concourse import bacc, bass_utils, mybir

B, C, H, W = 16, 256, 128, 128
N_CORES = 8
GROUP = 3          # images per PSUM bank / matmul group (130*3+2 fp32 <= 2KB bank)
STRIDE = 130       # 2-col zero gap + 128 data cols per image in the packed strip
PAD0 = 2           # upfirdn2d pad before (both spatial dims)
SG = 12            # images per DMA supergroup (4 matmul groups)
BF16 = ml_dtypes.bfloat16

# How the odd column shifts (d = j-2 = -1, +1) are realized:
#   "src": shift the moving-operand slice (odd bf16 SBUF start column)
#   "dst": shift the PSUM destination slice (odd fp32 PSUM start column)
SHIFT_MODE = os.environ.get("BLUR_SHIFT_MODE", "src")

_PROGRAM_CACHE: dict[tuple, object] = {}


def _band_matrices(kern: np.ndarray) -> np.ndarray:
    """bands[j][hi, ho] = wf[hi-ho+2, j], wf = flip(kern). Shape [4,128,128]."""
    wf = np.flip(np.asarray(kern, dtype=np.float64), (0, 1))
    bands = np.zeros((4, H, H), dtype=np.float64)
    ho = np.arange(H)
    for j in range(4):
        for i in range(4):
            d = i - PAD0            # hi - ho
            hi = ho + d
            m = (hi >= 0) & (hi < H)
            bands[j][hi[m], ho[m]] = wf[i, j]
    return bands


def _groups(n_images: int):
    """Split a supergroup's images into PSUM-bank groups of <= GROUP."""
    out = []
    i = 0
    while i < n_images:
        n = min(GROUP, n_images - i)
        out.append((i, n))
        i += n
    return out


def _supergroups(n_images: int):
    """Two tiny leading supergroups (fast PE warm-up: the first input DMAs
    land quickly) and a tiny trailing one (short copy+store tail); full-size
    SG in the middle."""
    out = []
    i = 0
    if n_images > 2 * SG:
        out = [(0, GROUP), (GROUP, GROUP)]
        i = 2 * GROUP
    while i < n_images:
        n = min(SG, n_images - i)
        # avoid a final full SG: split off a small trailing one
        if n == SG and n_images - i == SG:
            n = SG - 2
        out.append((i, n))
        i += n
    return out


def pack_input(x_imgs: np.ndarray) -> np.ndarray:
    """[n,128,128] fp32 -> gapped bf16 strip [128, (n+1)*130] (trailing
    block all zero; image k's data at cols [130k+2, 130k+130))."""
    n = x_imgs.shape[0]
    g = np.zeros((H, n + 1, STRIDE), dtype=BF16)
    g[:, :n, PAD0:] = x_imgs.transpose(1, 0, 2)
    return g.reshape(H, (n + 1) * STRIDE)


def build_program(n_images: int, shift_mode: str = SHIFT_MODE):
    nc = bacc.Bacc("TRN2", target_bir_lowering=False, debug=False)
    f32 = mybir.dt.float32
    bf16 = mybir.dt.bfloat16

    x_d = nc.dram_tensor("x", [H, (n_images + 1) * STRIDE], bf16, kind="ExternalInput")
    w_d = nc.dram_tensor("w", [H, 4 * H], bf16, kind="ExternalInput")
    y_d = nc.dram_tensor("y", [H, n_images * W], bf16, kind="ExternalOutput")

    sgs = _supergroups(n_images)

    with ExitStack() as ctx:
        tc = ctx.enter_context(tile.TileContext(nc))
        wpool = ctx.enter_context(tc.tile_pool(name="wpool", bufs=1))
        xpool = ctx.enter_context(tc.tile_pool(name="xpool", bufs=6))
        opool = ctx.enter_context(tc.tile_pool(name="opool", bufs=4))
        ppool = ctx.enter_context(tc.tile_pool(name="ppool", bufs=8, space="PSUM"))

        wt = wpool.tile([H, 4 * H], bf16)
        nc.sync.dma_start(wt, w_d[:, :])

        # PE warm-up: HAM throttles the array to K=4/8 (and low p-state) for
        # the first ~3.4us of activity. Burn that window on dummy matmuls
        # while the PE would otherwise idle waiting for the first DMAs, so
        # the real stream starts at full rate. gpsimd.memset gives an
        # initialized source with no DMA dependency.
        warm = wpool.tile([H, 3 * H], bf16, name="warm", tag="warm")
        nc.gpsimd.memset(warm, 0.0)
        warm_pt = ppool.tile([H, 2 * H], f32, tag="pt", name="warm_pt")
        n_warm = 12
        for i in range(n_warm):
            nc.tensor.matmul(
                warm_pt,
                warm[:, 0:H],
                warm[:, H : 3 * H],
                start=(i == 0),
                stop=(i == n_warm - 1),
            )

        xts: dict[int, object] = {}

        def in_dma(s):
            i0, n = sgs[s]
            xt = xpool.tile([H, SG * STRIDE + 2], bf16, name="xt", tag="xt")
            nc.sync.dma_start(
                xt[:, 0 : n * STRIDE + 2],
                x_d[:, i0 * STRIDE : i0 * STRIDE + n * STRIDE + 2],
            )
            xts[s] = xt

        for s0 in range(min(3, len(sgs))):
            in_dma(s0)
        for s, (i0, n) in enumerate(sgs):
            if s + 3 < len(sgs):
                in_dma(s + 3)
            xt = xts.pop(s)
            boff = 0
            gs = _groups(n)

            pts = [
                ppool.tile([H, ng * STRIDE + 2], f32, tag="pt", name="pt")
                for (g0, ng) in gs
            ]
            for idx, j in enumerate((2, 0, 1, 3)):
                d = j - PAD0
                for q, (g0, ng) in enumerate(gs):
                    base = (boff + g0) * STRIDE
                    if shift_mode == "src" or d in (0, -2):
                        a = PAD0
                        b = ng * STRIDE + PAD0 - (PAD0 if d > 0 else 0)
                        dst = pts[q][:, a:b]
                        src = xt[:, base + a + d : base + b + d]
                    elif d == -1:
                        dst = pts[q][:, 1 : ng * STRIDE + 1]
                        src = xt[:, base : base + ng * STRIDE]
                    else:  # d == +1
                        dst = pts[q][:, 1 : ng * STRIDE + 1]
                        src = xt[:, base + 2 : base + ng * STRIDE + 2]
                    nc.tensor.matmul(
                        dst,
                        wt[:, H * j : H * (j + 1)],
                        src,
                        start=(idx == 0),
                        stop=(idx == 3),
                    )

            # all of one SG's copies on a single engine (alternating per SG):
            # halves the PSUM-bank-free semaphore waits on the Tensor queue
            ot = opool.tile([H, SG * W], bf16, tag="ot", name="ot")
            for q, (g0, ng) in enumerate(gs):
                psrc = pts[q][:, 0 : ng * STRIDE].rearrange(
                    "p (k c) -> p k c", c=STRIDE
                )
                odst = ot[:, g0 * W : (g0 + ng) * W].rearrange(
                    "p (k c) -> p k c", c=W
                )
                if s % 2 == 0:
                    nc.vector.tensor_copy(odst, psrc[:, :, PAD0 : PAD0 + W])
                else:
                    nc.scalar.copy(odst, psrc[:, :, PAD0 : PAD0 + W])
            # final stores go out on the (idle by then) sync ring so they
            # don't queue behind earlier stores on scalar's FIFO ring
            oeng = nc.sync if s >= len(sgs) - 2 else nc.scalar
            oeng.dma_start(y_d[:, i0 * W : (i0 + n) * W], ot[:, 0 : n * W])

    nc.compile()
    return nc


def _get_program(n_images: int):
    key = (n_images, SHIFT_MODE)
    if key not in _PROGRAM_CACHE:
        _PROGRAM_CACHE[key] = build_program(n_images)
    return _PROGRAM_CACHE[key]


def kernel(x: np.ndarray, kernel: np.ndarray, _trace: bool = False):
    assert x.shape == (B, C, H, W), x.shape
    bands = _band_matrices(kernel)
    wt_host = np.concatenate(list(bands), axis=1).astype(BF16)  # [128, 512]

    n_total = B * C
    n_per_core = n_total // N_CORES
    xb = np.asarray(x, dtype=np.float32).reshape(n_total, H, W).astype(BF16)

    nc = _get_program(n_per_core)
    in_maps = [
        {
            "x": pack_input(xb[c * n_per_core : (c + 1) * n_per_core]),
            "w": wt_host,
        }
        for c in range(N_CORES)
    ]
    res = bass_utils.run_bass_kernel_spmd(
        nc, in_maps, core_ids=list(range(N_CORES)), trace=_trace
    )
    outs = [
        np.asarray(r["y"])
        .reshape(H, n_per_core, W)
        .transpose(1, 0, 2)
        .astype(np.float32)
        for r in res.results
    ]
    y = np.concatenate(outs, axis=0).reshape(B, C, H, W)
    if _trace:
        return y, res
    return y
